# revision 7
# baseline (speedup 1.0000x reference)
"""Block-sparse self-attention Trainium2 kernel (8 NeuronCores).

Strategy
--------
Shard by (batch, head-group): core c handles batch b = c//4 and heads
(c%4)*4 .. +4. Each core computes a partial output
sum_h softmax(mask(q_h k_h^T / 8)) v_h @ Wo[h-rows, :] for its 4 heads;
the host sums the 4 partials per batch.

Token layout (per batch, host-side): valid tokens are placed on a
per-block grid (block j's valid keys start at >= j*18) so both batches'
key layouts align; invalid tokens follow from position NVP, sorted by
block. Keys are the first NVP positions only, so K/V projections run on
~NVP=1152 columns instead of 2048, and each query tile's same-block
keys lie in a short window of the key prefix. Per-tile window starts /
widths (128-aligned, 256 or 384) come from the actual block assignment
at build time; the module cache is keyed on them.

Masking is folded into the score matmul: rows 64..127 of the qT/kT
operands hold 8*onehot(q) and 16*onehot(k)(valid-only), so the K=128
score matmul computes q.k/8 + 128*[same block & key valid], and
exp(s - 144) underflows invalid pairs to exactly 0.

Engine budget (the whole point of this structure): inputs stream bf16
(DMA is a serialized resource); exp(+accumulated denominators) on Act;
softmax normalize + q/k head-splits on the otherwise idle Pool engine;
the [q,k]->[k,q] transpose for AV runs on the DMA XBAR
(dma_start_transpose), one instruction per query tile for all 4 heads,
eliminating 180 PE transposes and all PSUM round-trips for it; Wo runs
f32r (self-loading matmuls - no Ldweights dispatch). Emission is
software-pipelined: scores/exp/transpose of tile i issue ~3 tiles ahead
of AV/Wo of tile i so the XBAR latency hides behind PE work.
"""

import numpy as np
import ml_dtypes
from contextlib import ExitStack

import concourse.bass as bass
from concourse import bacc
import concourse.mybir as mybir
import concourse.tile as tile
from concourse.bass_utils import run_bass_kernel_spmd
from concourse.masks import make_identity

B = 2
N = 2048
UNITS = 1024
H = 16
HD = 64
NB = 64
NCORES = 8
HPC = 4            # heads per core
CPB = 4            # cores per batch
NKT = UNITS // 128  # 8 contraction tiles for projections
GRID = 18          # valid-key grid stride per block
LAG = 3            # stage-B emission lag (tiles) to hide XBAR latency
EXP_BIAS = -144.0  # -(128 mask offset + 16 score headroom)

F32 = mybir.dt.float32
F32R = mybir.dt.float32r
BF16 = mybir.dt.bfloat16
BF16NP = ml_dtypes.bfloat16

_CACHE = {}
_LAST_NC = None


def _build(nq: int, nvp: int, windows: tuple):
    """windows: per query-tile (t0, w); window keys are
    [t0*128, t0*128+w) of the NVP key prefix, w in {256, 384}."""
    ntq = nq // 128
    ntk = nvp // 128
    assert len(windows) == ntq
    nc = bacc.Bacc("TRN2", target_bir_lowering=False, debug=False)

    xts = nc.dram_tensor("xts", [128, NKT, nq], BF16, kind="ExternalInput")
    wq = nc.dram_tensor("wq", [128, NKT, HPC * HD], BF16, kind="ExternalInput")
    wk = nc.dram_tensor("wk", [128, NKT, HPC * HD], BF16, kind="ExternalInput")
    wv = nc.dram_tensor("wv", [128, NKT, HPC * HD], BF16, kind="ExternalInput")
    wo = nc.dram_tensor("wo", [128, 2, UNITS], F32R, kind="ExternalInput")
    augq = nc.dram_tensor("augq", [NB, HPC, nq], BF16, kind="ExternalInput")
    augk = nc.dram_tensor("augk", [NB, HPC, nvp], BF16, kind="ExternalInput")
    out = nc.dram_tensor("out", [nq, UNITS], BF16, kind="ExternalOutput")

    chunks = []
    c0 = 0
    while c0 < nq:
        cw = min(512, nq - c0)
        chunks.append((c0, cw))
        c0 += cw

    def attn_ready(i, cols):
        t0, w = windows[i]
        return (i + 1) * 128 <= cols and t0 * 128 + w <= cols

    with tile.TileContext(nc) as tc, ExitStack() as ctx:
        persist = ctx.enter_context(tc.tile_pool(name="persist", bufs=1))

        # ---- persistent tensors; DMA emission order = queue drain order
        wq_sb = persist.tile([128, NKT, HPC * HD], BF16, tag="wq_sb")
        nc.sync.dma_start(wq_sb[:], wq.ap())
        wk_sb = persist.tile([128, NKT, HPC * HD], BF16, tag="wk_sb")
        nc.sync.dma_start(wk_sb[:], wk.ap())
        x_sb = persist.tile([128, NKT, nq], BF16, tag="x_sb")
        for kt in range(NKT):
            nc.sync.dma_start(x_sb[:, kt, 0:512], xts.ap()[:, kt, 0:512])
        wv_sb = persist.tile([128, NKT, HPC * HD], BF16, tag="wv_sb")
        nc.sync.dma_start(wv_sb[:], wv.ap())
        for kt in range(NKT):
            nc.sync.dma_start(x_sb[:, kt, 512:1024], xts.ap()[:, kt, 512:1024])

        qT_sb = persist.tile([128, HPC, nq], BF16, tag="qT_sb")
        kT_sb = persist.tile([128, HPC, nvp], BF16, tag="kT_sb")
        v_sb = persist.tile([128, ntk, HPC * HD], BF16, tag="v_sb")
        wo_sb = persist.tile([128, 2, UNITS], F32R, tag="wo_sb")
        nc.sync.dma_start(qT_sb[64:128, :, :], augq.ap())
        nc.sync.dma_start(kT_sb[64:128, :, :], augk.ap())
        nc.sync.dma_start(wo_sb[:], wo.ap())
        for kt in range(NKT):
            nc.sync.dma_start(x_sb[:, kt, 1024:nq], xts.ap()[:, kt, 1024:nq])

        bias_t = persist.tile([128, 1], F32, tag="bias_t")
        nc.vector.memset(bias_t[:], EXP_BIAS)

        stage = ctx.enter_context(tc.tile_pool(name="stage", bufs=3))
        ewpool = ctx.enter_context(tc.tile_pool(name="ewpool", bufs=3))
        ewtpool = ctx.enter_context(tc.tile_pool(name="ewtpool", bufs=3))
        spool = ctx.enter_context(tc.tile_pool(name="spool", bufs=5))
        otpool = ctx.enter_context(tc.tile_pool(name="otpool", bufs=2))
        stpool = ctx.enter_context(tc.tile_pool(name="stpool", bufs=2))
        pp_proj = ctx.enter_context(
            tc.tile_pool(name="pp_proj", bufs=2, space="PSUM"))
        pp_s = ctx.enter_context(tc.tile_pool(name="pp_s", bufs=2, space="PSUM"))
        pp_av = ctx.enter_context(tc.tile_pool(name="pp_av", bufs=2, space="PSUM"))
        pp_f = ctx.enter_context(tc.tile_pool(name="pp_f", bufs=1, space="PSUM"))

        pend_a = {}
        pend = {}

        def emit_stage_a(i):
            t0, w = windows[i]
            qs = slice(i * 128, (i + 1) * 128)
            ks = slice(t0 * 128, t0 * 128 + w)
            stats = spool.tile([128, HPC], F32, tag="stats")
            ew_all = ewpool.tile([128, HPC, w], BF16, tag=f"ew{w}")
            for h in range(HPC):
                s_ps = pp_s.tile([128, 384], F32, tag="s_ps")
                nc.tensor.matmul(s_ps[:, 0:w], qT_sb[:, h, qs],
                                 kT_sb[:, h, ks], start=True, stop=True)
                nc.scalar.activation(ew_all[:, h, :], s_ps[:, 0:w],
                                     mybir.ActivationFunctionType.Exp,
                                     bias=bias_t[:], scale=1.0,
                                     accum_out=stats[:, h:h + 1])
            pend_a[i] = (stats, ew_all)

        def emit_stage_m(i):
            t0, w = windows[i]
            nch = w // 128
            stats, ew_all = pend_a.pop(i)
            r_t = spool.tile([128, HPC], F32, tag="r_t")
            nc.vector.reciprocal(r_t[:], stats[:])
            for h in range(HPC):
                nc.gpsimd.tensor_scalar_mul(ew_all[:, h, :], ew_all[:, h, :],
                                            r_t[:, h:h + 1])
            ewt = ewtpool.tile([128, HPC * nch, 128], BF16, tag=f"ewt{w}")
            nc.scalar.dma_start_transpose(ewt[:], ew_all[:])
            pend[i] = ewt

        def emit_stage_b(i):
            t0, w = windows[i]
            nch = w // 128
            qs = slice(i * 128, (i + 1) * 128)
            ewt = pend.pop(i)
            av2 = pp_av.tile([128, 2, 128], F32, tag="av2")
            for hp in range(2):
                for s in range(2):
                    h = 2 * hp + s
                    for j in range(nch):
                        nc.tensor.matmul(
                            av2[s * 64:(s + 1) * 64, hp, :],
                            v_sb[:, t0 + j, h * HD:(h + 1) * HD],
                            ewt[:, h * nch + j, :],
                            start=(j == 0), stop=(j == nch - 1))
            outT = otpool.tile([128, 2, 128], F32R, tag="outT")
            nc.vector.tensor_copy(outT[:], av2[:])
            st = stpool.tile([128, UNITS], BF16, tag="st")
            f_ps = pp_f.tile([128, UNITS], F32, tag="f_ps")
            for fc2 in range(2):
                for mt in range(2):
                    nc.tensor.matmul(
                        f_ps[:, fc2 * 512:(fc2 + 1) * 512], outT[:, mt, :],
                        wo_sb[:, mt, fc2 * 512:(fc2 + 1) * 512],
                        start=(mt == 0), stop=(mt == 1))
            nc.vector.tensor_copy(st[:], f_ps[:])
            nc.sync.dma_start(out.ap()[qs, :], st[:])

        # ---- projection sweep with pipelined attention interleaved ----
        a_done = 0
        m_done = 0
        b_done = 0

        def advance():
            nonlocal m_done, b_done
            while m_done < a_done - 1:
                emit_stage_m(m_done)
                m_done += 1
            while b_done < a_done - LAG:
                emit_stage_b(b_done)
                b_done += 1

        for (c0, cw) in chunks:
            cols_k = min(max(nvp - c0, 0), cw)
            for dst, w_sb, cw_d in ((qT_sb, wq_sb, cw), (kT_sb, wk_sb, cols_k)):
                if cw_d == 0:
                    continue
                for m in range(2):
                    ps = pp_proj.tile([128, 512], F32, tag="ps")
                    for kt in range(NKT):
                        nc.tensor.matmul(
                            ps[:, 0:cw_d], w_sb[:, kt, m * 128:(m + 1) * 128],
                            x_sb[:, kt, c0:c0 + cw_d],
                            start=(kt == 0), stop=(kt == NKT - 1))
                    stg = stage.tile([128, 512], BF16, tag="stg")
                    nc.vector.tensor_copy(stg[:, 0:cw_d], ps[:, 0:cw_d])
                    # head split (SBUF->SBUF): q on Pool, k on Act
                    for s in range(2):
                        h = 2 * m + s
                        src = stg[s * 64:(s + 1) * 64, 0:cw_d]
                        if dst is qT_sb:
                            nc.gpsimd.tensor_copy(dst[0:64, h, c0:c0 + cw_d], src)
                        else:
                            nc.scalar.copy(dst[0:64, h, c0:c0 + cw_d], src)
            for qi in range(cols_k // 128):
                ps = pp_proj.tile([128, 512], F32, tag="ps")
                for kt in range(NKT):
                    nc.tensor.matmul(
                        ps[:, 0:HPC * HD],
                        x_sb[:, kt, c0 + qi * 128:c0 + (qi + 1) * 128],
                        wv_sb[:, kt, :], start=(kt == 0), stop=(kt == NKT - 1))
                nc.vector.tensor_copy(v_sb[:, c0 // 128 + qi, :],
                                      ps[:, 0:HPC * HD])
            cols = c0 + cw
            while a_done < ntq and attn_ready(a_done, cols):
                emit_stage_a(a_done)
                a_done += 1
                advance()
        while a_done < ntq:
            emit_stage_a(a_done)
            a_done += 1
            advance()
        while m_done < ntq:
            emit_stage_m(m_done)
            m_done += 1
            advance()
        while b_done < ntq:
            emit_stage_b(b_done)
            b_done += 1

    nc.compile()
    return nc


def _get_nc(key=None):
    global _LAST_NC
    if key is None:
        return _LAST_NC
    if key not in _CACHE:
        _CACHE[key] = _build(*key)
    _LAST_NC = _CACHE[key]
    return _CACHE[key]


def _plan(blockB, NodalMask):
    """Grid-aligned token layout + per-tile key windows (shared across
    batches)."""
    bids = [np.argmax(blockB[b], -1) for b in range(B)]
    nodal = [NodalMask[b] != 0 for b in range(B)]
    nv = [int(nodal[b].sum()) for b in range(B)]

    starts = []     # per batch: [NB] placed start of each block's valid keys
    ends = []
    pos = []        # per batch: layout position of each token
    vend_max = 0
    for b in range(B):
        counts = np.bincount(bids[b][nodal[b]], minlength=NB)
        st = np.zeros(NB, np.int64)
        en = np.zeros(NB, np.int64)
        p = 0
        for j in range(NB):
            p = max(p, j * GRID)
            st[j] = p
            p += counts[j]
            en[j] = p
        starts.append(st)
        ends.append(en)
        vend_max = max(vend_max, int(en[-1]))
    nvp = -(-vend_max // 128) * 128
    ni_max = max(N - v for v in nv)
    nq = nvp + (-(-ni_max // 128) * 128)

    for b in range(B):
        p = np.full(N, -1, np.int64)
        order_v = np.argsort(np.where(nodal[b], bids[b], NB + 1),
                             kind="stable")[: nv[b]]
        cnt = np.zeros(NB, np.int64)
        for tok in order_v:
            j = bids[b][tok]
            p[tok] = starts[b][j] + cnt[j]
            cnt[j] += 1
        order_i = np.argsort(np.where(~nodal[b], bids[b], NB + 1),
                             kind="stable")[: N - nv[b]]
        p[order_i] = nvp + np.arange(N - nv[b])
        pos.append(p)

    windows = []
    for i in range(nq // 128):
        lo, hi = nvp, 0
        for b in range(B):
            in_tile = (pos[b] >= i * 128) & (pos[b] < (i + 1) * 128)
            if not in_tile.any():
                continue
            blk = bids[b][in_tile]
            lo = min(lo, int(starts[b][blk.min()]))
            hi = max(hi, int(ends[b][blk.max()]))
        if hi <= lo:
            windows.append((0, 256))
            continue
        t0 = lo // 128
        w = 256 if hi <= t0 * 128 + 256 else 384
        t0 = min(t0, nvp // 128 - w // 128)
        assert lo >= t0 * 128 and hi <= t0 * 128 + w, (i, lo, hi, t0, w)
        windows.append((t0, w))
    return nq, nvp, tuple(windows), pos


def kernel(x, blockB, NodalMask, Wq, Wk, Wv, Wo):
    x = np.asarray(x, dtype=np.float32)
    blockB = np.asarray(blockB, dtype=np.float32)
    NodalMask = np.asarray(NodalMask, dtype=np.float32)
    Wq = np.asarray(Wq, dtype=np.float32)
    Wk = np.asarray(Wk, dtype=np.float32)
    Wv = np.asarray(Wv, dtype=np.float32)
    Wo = np.asarray(Wo, dtype=np.float32)

    nq, nvp, windows, pos = _plan(blockB, NodalMask)

    batch_data = []
    for b in range(B):
        p = pos[b]
        valid = NodalMask[b] != 0
        xs = np.zeros((nq, UNITS), np.float32)
        xs[p] = x[b]
        xts = np.ascontiguousarray(
            xs.T.reshape(NKT, 128, nq).transpose(1, 0, 2)).astype(BF16NP)
        aq = np.zeros((NB, nq), np.float32)
        aq[:, p] = 8.0 * blockB[b].T
        ak = np.zeros((NB, nvp), np.float32)
        ak[:, p[valid]] = 16.0 * blockB[b][valid].T
        augq = np.ascontiguousarray(
            np.broadcast_to(aq[:, None, :], (NB, HPC, nq))).astype(BF16NP)
        augk = np.ascontiguousarray(
            np.broadcast_to(ak[:, None, :], (NB, HPC, nvp))).astype(BF16NP)
        batch_data.append((xts, augq, augk))

    in_maps = []
    for c in range(NCORES):
        b, hg = c // CPB, c % CPB
        xts, augq, augk = batch_data[b]
        cols = slice(hg * HPC * HD, (hg + 1) * HPC * HD)
        wq_h = (Wq[:, cols] * 0.125).reshape(NKT, 128, HPC * HD)
        wk_h = Wk[:, cols].reshape(NKT, 128, HPC * HD)
        wv_h = Wv[:, cols].reshape(NKT, 128, HPC * HD)
        wo_h = Wo[cols, :].reshape(2, 128, UNITS)
        in_maps.append({
            "xts": xts,
            "wq": np.ascontiguousarray(wq_h.transpose(1, 0, 2)).astype(BF16NP),
            "wk": np.ascontiguousarray(wk_h.transpose(1, 0, 2)).astype(BF16NP),
            "wv": np.ascontiguousarray(wv_h.transpose(1, 0, 2)).astype(BF16NP),
            "wo": np.ascontiguousarray(wo_h.transpose(1, 0, 2)),
            "augq": augq,
            "augk": augk,
        })

    nc = _get_nc((nq, nvp, windows))
    res = run_bass_kernel_spmd(nc, in_maps, core_ids=list(range(NCORES)))

    result = np.empty((B, N, UNITS), dtype=np.float32)
    for b in range(B):
        acc = res.results[b * CPB]["out"].astype(np.float32)
        for hg in range(1, CPB):
            acc = acc + res.results[b * CPB + hg]["out"].astype(np.float32)
        result[b] = acc[pos[b]]
    return result


# revision 10
# speedup vs baseline: 1.0476x; 1.0476x over previous
"""Block-sparse self-attention Trainium2 kernel (8 NeuronCores).

Strategy
--------
Shard by (batch, head-group): core c handles batch b = c//4 and heads
(c%4)*4 .. +4. Each core computes a partial output
sum_h softmax(mask(q_h k_h^T / 8)) v_h @ Wo[h-rows, :] for its 4 heads;
the host sums the 4 partials per batch.

Token layout (per batch, host-side): valid tokens are placed on a
per-block grid (block j's valid keys start at >= j*18) so both batches'
key layouts align; invalid tokens follow from position NVP, sorted by
block. Keys are the first NVP positions only, so K/V projections run on
~NVP=1152 columns instead of 2048, and each query tile's same-block
keys lie in a short window of the key prefix. Per-tile window starts /
widths (128-aligned, 256 or 384) come from the actual block assignment
at build time; the module cache is keyed on them.

Masking is folded into the score matmul: rows 64..127 of the qT/kT
operands hold 8*onehot(q) and 16*onehot(k)(valid-only), so the K=128
score matmul computes q.k/8 + 128*[same block & key valid], and
exp(s - 144) underflows invalid pairs to exactly 0.

Engine budget (the whole point of this structure): inputs stream bf16
(DMA is a serialized resource); exp(+accumulated denominators) on Act;
softmax normalize + q/k head-splits on the otherwise idle Pool engine;
the [q,k]->[k,q] transpose for AV runs on the DMA XBAR
(dma_start_transpose), one instruction per query tile for all 4 heads,
eliminating 180 PE transposes and all PSUM round-trips for it; Wo runs
f32r (self-loading matmuls - no Ldweights dispatch). Emission is
software-pipelined: scores/exp/transpose of tile i issue ~3 tiles ahead
of AV/Wo of tile i so the XBAR latency hides behind PE work.
"""

import numpy as np
import ml_dtypes
from contextlib import ExitStack

import concourse.bass as bass
from concourse import bacc
import concourse.mybir as mybir
import concourse.tile as tile
from concourse.bass_utils import run_bass_kernel_spmd
from concourse.masks import make_identity

B = 2
N = 2048
UNITS = 1024
H = 16
HD = 64
NB = 64
NCORES = 8
HPC = 4            # heads per core
CPB = 4            # cores per batch
NKT = UNITS // 128  # 8 contraction tiles for projections
GRID = 18          # valid-key grid stride per block
MLAG = 2           # normalize+transpose emission lag (tiles)
LAG = 4            # AV/Wo emission lag (tiles) to hide XBAR latency
EXP_BIAS = -144.0  # -(128 mask offset + 16 score headroom)

F32 = mybir.dt.float32
F32R = mybir.dt.float32r
BF16 = mybir.dt.bfloat16
BF16NP = ml_dtypes.bfloat16

_CACHE = {}
_LAST_NC = None


def _build(nq: int, nvp: int, windows: tuple):
    """windows: per query-tile (t0, w); window keys are
    [t0*128, t0*128+w) of the NVP key prefix, w in {256, 384}."""
    ntq = nq // 128
    ntk = nvp // 128
    assert len(windows) == ntq
    nc = bacc.Bacc("TRN2", target_bir_lowering=False, debug=False)

    xts = nc.dram_tensor("xts", [128, NKT, nq], BF16, kind="ExternalInput")
    wq = nc.dram_tensor("wq", [128, NKT, HPC * HD], BF16, kind="ExternalInput")
    wk = nc.dram_tensor("wk", [128, NKT, HPC * HD], BF16, kind="ExternalInput")
    wv = nc.dram_tensor("wv", [128, NKT, HPC * HD], BF16, kind="ExternalInput")
    wo = nc.dram_tensor("wo", [128, 2, UNITS], F32R, kind="ExternalInput")
    augq = nc.dram_tensor("augq", [NB, HPC, nq], BF16, kind="ExternalInput")
    augk = nc.dram_tensor("augk", [NB, HPC, nvp], BF16, kind="ExternalInput")
    out = nc.dram_tensor("out", [nq, UNITS], BF16, kind="ExternalOutput")

    chunks = []
    c0 = 0
    while c0 < nq:
        cw = min(512, nq - c0)
        chunks.append((c0, cw))
        c0 += cw

    def attn_ready(i, cols):
        t0, w = windows[i]
        return (i + 1) * 128 <= cols and t0 * 128 + w <= cols

    with tile.TileContext(nc) as tc, ExitStack() as ctx:
        persist = ctx.enter_context(tc.tile_pool(name="persist", bufs=1))

        # ---- persistent tensors; DMA emission order = queue drain order
        wq_sb = persist.tile([128, NKT, HPC * HD], BF16, tag="wq_sb")
        nc.sync.dma_start(wq_sb[:], wq.ap())
        wk_sb = persist.tile([128, NKT, HPC * HD], BF16, tag="wk_sb")
        nc.sync.dma_start(wk_sb[:], wk.ap())
        x_sb = persist.tile([128, NKT, nq], BF16, tag="x_sb")
        for kt in range(NKT):
            nc.sync.dma_start(x_sb[:, kt, 0:512], xts.ap()[:, kt, 0:512])
        wv_sb = persist.tile([128, NKT, HPC * HD], BF16, tag="wv_sb")
        nc.sync.dma_start(wv_sb[:], wv.ap())
        for kt in range(NKT):
            nc.sync.dma_start(x_sb[:, kt, 512:1024], xts.ap()[:, kt, 512:1024])

        qT_sb = persist.tile([128, HPC, nq], BF16, tag="qT_sb")
        kT_sb = persist.tile([128, HPC, nvp], BF16, tag="kT_sb")
        v_sb = persist.tile([128, ntk, HPC * HD], BF16, tag="v_sb")
        wo_sb = persist.tile([128, 2, UNITS], F32R, tag="wo_sb")
        nc.sync.dma_start(qT_sb[64:128, :, :], augq.ap())
        nc.sync.dma_start(kT_sb[64:128, :, :], augk.ap())
        nc.sync.dma_start(wo_sb[:], wo.ap())
        for kt in range(NKT):
            nc.sync.dma_start(x_sb[:, kt, 1024:nq], xts.ap()[:, kt, 1024:nq])

        bias_t = persist.tile([128, 1], F32, tag="bias_t")
        nc.vector.memset(bias_t[:], EXP_BIAS)

        stage = ctx.enter_context(tc.tile_pool(name="stage", bufs=3))
        ewpool = ctx.enter_context(tc.tile_pool(name="ewpool", bufs=4))
        ewtpool = ctx.enter_context(tc.tile_pool(name="ewtpool", bufs=3))
        spool = ctx.enter_context(tc.tile_pool(name="spool", bufs=6))
        otpool = ctx.enter_context(tc.tile_pool(name="otpool", bufs=2))
        stpool = ctx.enter_context(tc.tile_pool(name="stpool", bufs=3))
        pp_proj = ctx.enter_context(
            tc.tile_pool(name="pp_proj", bufs=2, space="PSUM"))
        pp_s = ctx.enter_context(tc.tile_pool(name="pp_s", bufs=2, space="PSUM"))
        pp_av = ctx.enter_context(tc.tile_pool(name="pp_av", bufs=2, space="PSUM"))
        pp_f = ctx.enter_context(tc.tile_pool(name="pp_f", bufs=1, space="PSUM"))

        pend_a = {}
        pend = {}

        def emit_stage_a(i):
            t0, w = windows[i]
            qs = slice(i * 128, (i + 1) * 128)
            ks = slice(t0 * 128, t0 * 128 + w)
            stats = spool.tile([128, HPC], F32, tag="stats")
            ew_all = ewpool.tile([128, HPC, w], BF16, tag=f"ew{w}")
            for h in range(HPC):
                s_ps = pp_s.tile([128, 384], F32, tag="s_ps")
                nc.tensor.matmul(s_ps[:, 0:w], qT_sb[:, h, qs],
                                 kT_sb[:, h, ks], start=True, stop=True)
                nc.scalar.activation(ew_all[:, h, :], s_ps[:, 0:w],
                                     mybir.ActivationFunctionType.Exp,
                                     bias=bias_t[:], scale=1.0,
                                     accum_out=stats[:, h:h + 1])
            pend_a[i] = (stats, ew_all)

        def emit_stage_m(i):
            t0, w = windows[i]
            nch = w // 128
            stats, ew_all = pend_a.pop(i)
            r_t = spool.tile([128, HPC], F32, tag="r_t")
            nc.vector.reciprocal(r_t[:], stats[:])
            for h in range(HPC):
                nc.gpsimd.tensor_scalar_mul(ew_all[:, h, :], ew_all[:, h, :],
                                            r_t[:, h:h + 1])
            ewt = ewtpool.tile([128, HPC * nch, 128], BF16, tag=f"ewt{w}")
            nc.scalar.dma_start_transpose(ewt[:], ew_all[:])
            pend[i] = ewt

        def emit_stage_b(i):
            t0, w = windows[i]
            nch = w // 128
            qs = slice(i * 128, (i + 1) * 128)
            ewt = pend.pop(i)
            av2 = pp_av.tile([128, 2, 128], F32, tag="av2")
            for hp in range(2):
                for s in range(2):
                    h = 2 * hp + s
                    for j in range(nch):
                        nc.tensor.matmul(
                            av2[s * 64:(s + 1) * 64, hp, :],
                            v_sb[:, t0 + j, h * HD:(h + 1) * HD],
                            ewt[:, h * nch + j, :],
                            start=(j == 0), stop=(j == nch - 1))
            outT = otpool.tile([128, 2, 128], F32R, tag="outT")
            nc.vector.tensor_copy(outT[:], av2[:])
            st = stpool.tile([128, UNITS], BF16, tag="st")
            f_ps = pp_f.tile([128, UNITS], F32, tag="f_ps")
            for fc2 in range(2):
                for mt in range(2):
                    nc.tensor.matmul(
                        f_ps[:, fc2 * 512:(fc2 + 1) * 512], outT[:, mt, :],
                        wo_sb[:, mt, fc2 * 512:(fc2 + 1) * 512],
                        start=(mt == 0), stop=(mt == 1))
            nc.vector.tensor_copy(st[:], f_ps[:])
            nc.sync.dma_start(out.ap()[qs, :], st[:])

        # ---- projection sweep with pipelined attention interleaved ----
        a_done = 0
        m_done = 0
        b_done = 0

        def advance():
            nonlocal m_done, b_done
            while m_done < a_done - MLAG:
                emit_stage_m(m_done)
                m_done += 1
            while b_done < a_done - LAG:
                emit_stage_b(b_done)
                b_done += 1

        for (c0, cw) in chunks:
            cols_k = min(max(nvp - c0, 0), cw)
            for dst, w_sb, cw_d in ((qT_sb, wq_sb, cw), (kT_sb, wk_sb, cols_k)):
                if cw_d == 0:
                    continue
                for m in range(2):
                    ps = pp_proj.tile([128, 512], F32, tag="ps")
                    for kt in range(NKT):
                        nc.tensor.matmul(
                            ps[:, 0:cw_d], w_sb[:, kt, m * 128:(m + 1) * 128],
                            x_sb[:, kt, c0:c0 + cw_d],
                            start=(kt == 0), stop=(kt == NKT - 1))
                    stg = stage.tile([128, 512], BF16, tag="stg")
                    nc.vector.tensor_copy(stg[:, 0:cw_d], ps[:, 0:cw_d])
                    # head split (SBUF->SBUF): q on Pool, k on Act
                    for s in range(2):
                        h = 2 * m + s
                        src = stg[s * 64:(s + 1) * 64, 0:cw_d]
                        if dst is qT_sb:
                            nc.gpsimd.tensor_copy(dst[0:64, h, c0:c0 + cw_d], src)
                        else:
                            nc.scalar.copy(dst[0:64, h, c0:c0 + cw_d], src)
            for qi in range(cols_k // 128):
                ps = pp_proj.tile([128, 512], F32, tag="ps")
                for kt in range(NKT):
                    nc.tensor.matmul(
                        ps[:, 0:HPC * HD],
                        x_sb[:, kt, c0 + qi * 128:c0 + (qi + 1) * 128],
                        wv_sb[:, kt, :], start=(kt == 0), stop=(kt == NKT - 1))
                nc.vector.tensor_copy(v_sb[:, c0 // 128 + qi, :],
                                      ps[:, 0:HPC * HD])
            cols = c0 + cw
            while a_done < ntq and attn_ready(a_done, cols):
                emit_stage_a(a_done)
                a_done += 1
                advance()
        while a_done < ntq:
            emit_stage_a(a_done)
            a_done += 1
            advance()
        while m_done < ntq:
            emit_stage_m(m_done)
            m_done += 1
            advance()
        while b_done < ntq:
            emit_stage_b(b_done)
            b_done += 1

    nc.compile()
    return nc


def _get_nc(key=None):
    global _LAST_NC
    if key is None:
        return _LAST_NC
    if key not in _CACHE:
        _CACHE[key] = _build(*key)
    _LAST_NC = _CACHE[key]
    return _CACHE[key]


def _plan(blockB, NodalMask):
    """Grid-aligned token layout + per-tile key windows (shared across
    batches)."""
    bids = [np.argmax(blockB[b], -1) for b in range(B)]
    nodal = [NodalMask[b] != 0 for b in range(B)]
    nv = [int(nodal[b].sum()) for b in range(B)]

    starts = []     # per batch: [NB] placed start of each block's valid keys
    ends = []
    pos = []        # per batch: layout position of each token
    vend_max = 0
    for b in range(B):
        counts = np.bincount(bids[b][nodal[b]], minlength=NB)
        st = np.zeros(NB, np.int64)
        en = np.zeros(NB, np.int64)
        p = 0
        for j in range(NB):
            p = max(p, j * GRID)
            st[j] = p
            p += counts[j]
            en[j] = p
        starts.append(st)
        ends.append(en)
        vend_max = max(vend_max, int(en[-1]))
    nvp = -(-vend_max // 128) * 128
    ni_max = max(N - v for v in nv)
    nq = nvp + (-(-ni_max // 128) * 128)

    for b in range(B):
        p = np.full(N, -1, np.int64)
        order_v = np.argsort(np.where(nodal[b], bids[b], NB + 1),
                             kind="stable")[: nv[b]]
        cnt = np.zeros(NB, np.int64)
        for tok in order_v:
            j = bids[b][tok]
            p[tok] = starts[b][j] + cnt[j]
            cnt[j] += 1
        order_i = np.argsort(np.where(~nodal[b], bids[b], NB + 1),
                             kind="stable")[: N - nv[b]]
        p[order_i] = nvp + np.arange(N - nv[b])
        pos.append(p)

    windows = []
    for i in range(nq // 128):
        lo, hi = nvp, 0
        for b in range(B):
            in_tile = (pos[b] >= i * 128) & (pos[b] < (i + 1) * 128)
            if not in_tile.any():
                continue
            blk = bids[b][in_tile]
            lo = min(lo, int(starts[b][blk.min()]))
            hi = max(hi, int(ends[b][blk.max()]))
        if hi <= lo:
            windows.append((0, 256))
            continue
        t0 = lo // 128
        w = 256 if hi <= t0 * 128 + 256 else 384
        t0 = min(t0, nvp // 128 - w // 128)
        assert lo >= t0 * 128 and hi <= t0 * 128 + w, (i, lo, hi, t0, w)
        windows.append((t0, w))
    return nq, nvp, tuple(windows), pos


def kernel(x, blockB, NodalMask, Wq, Wk, Wv, Wo):
    x = np.asarray(x, dtype=np.float32)
    blockB = np.asarray(blockB, dtype=np.float32)
    NodalMask = np.asarray(NodalMask, dtype=np.float32)
    Wq = np.asarray(Wq, dtype=np.float32)
    Wk = np.asarray(Wk, dtype=np.float32)
    Wv = np.asarray(Wv, dtype=np.float32)
    Wo = np.asarray(Wo, dtype=np.float32)

    nq, nvp, windows, pos = _plan(blockB, NodalMask)

    batch_data = []
    for b in range(B):
        p = pos[b]
        valid = NodalMask[b] != 0
        xs = np.zeros((nq, UNITS), np.float32)
        xs[p] = x[b]
        xts = np.ascontiguousarray(
            xs.T.reshape(NKT, 128, nq).transpose(1, 0, 2)).astype(BF16NP)
        aq = np.zeros((NB, nq), np.float32)
        aq[:, p] = 8.0 * blockB[b].T
        ak = np.zeros((NB, nvp), np.float32)
        ak[:, p[valid]] = 16.0 * blockB[b][valid].T
        augq = np.ascontiguousarray(
            np.broadcast_to(aq[:, None, :], (NB, HPC, nq))).astype(BF16NP)
        augk = np.ascontiguousarray(
            np.broadcast_to(ak[:, None, :], (NB, HPC, nvp))).astype(BF16NP)
        batch_data.append((xts, augq, augk))

    in_maps = []
    for c in range(NCORES):
        b, hg = c // CPB, c % CPB
        xts, augq, augk = batch_data[b]
        cols = slice(hg * HPC * HD, (hg + 1) * HPC * HD)
        wq_h = (Wq[:, cols] * 0.125).reshape(NKT, 128, HPC * HD)
        wk_h = Wk[:, cols].reshape(NKT, 128, HPC * HD)
        wv_h = Wv[:, cols].reshape(NKT, 128, HPC * HD)
        wo_h = Wo[cols, :].reshape(2, 128, UNITS)
        in_maps.append({
            "xts": xts,
            "wq": np.ascontiguousarray(wq_h.transpose(1, 0, 2)).astype(BF16NP),
            "wk": np.ascontiguousarray(wk_h.transpose(1, 0, 2)).astype(BF16NP),
            "wv": np.ascontiguousarray(wv_h.transpose(1, 0, 2)).astype(BF16NP),
            "wo": np.ascontiguousarray(wo_h.transpose(1, 0, 2)),
            "augq": augq,
            "augk": augk,
        })

    nc = _get_nc((nq, nvp, windows))
    res = run_bass_kernel_spmd(nc, in_maps, core_ids=list(range(NCORES)))

    result = np.empty((B, N, UNITS), dtype=np.float32)
    for b in range(B):
        acc = res.results[b * CPB]["out"].astype(np.float32)
        for hg in range(1, CPB):
            acc = acc + res.results[b * CPB + hg]["out"].astype(np.float32)
        result[b] = acc[pos[b]]
    return result
